# revision 3
# baseline (speedup 1.0000x reference)
"""Trainium2 Bass kernel for nn_AnswerSelection (dense MLP 600->75->relu->1).

Strategy: pure data parallel across 8 NeuronCores — shard the batch dim of
answer_vector, replicate the tiny weights. Per core: 62500 rows x 600 feats.

Per-core pipeline (groups of 256 rows):
  DMA in (natural [row, feat] layout, full-bandwidth contiguous reads)
  -> PE transpose via identity matmul (f32), putting features on partitions
  -> ACT/DVE copyback from PSUM with cast to bf16
  -> 5 accumulating bf16 matmuls against W1^T chunks (K=120 each) -> h^T PSUM
  -> fused bias+relu on ACT (PSUM -> SBUF bf16)
  -> 1-column matmul against W2^T -> scores PSUM
  -> staged score buffer -> DMA out.
b2 is added on the host (scalar).
"""

import sys

if "/opt/trn_rl_repo" not in sys.path:
    sys.path.insert(0, "/opt/trn_rl_repo")

import numpy as np

import concourse.bacc as bacc
import concourse.tile as tile
from concourse import mybir
from concourse.bass_utils import run_bass_kernel_spmd
from concourse.masks import make_identity

N_CORES = 8
OPTIONS = 5
BATCH = 100000
BSHARD = BATCH // N_CORES  # 12500
ROWS = OPTIONS * BSHARD  # 62500
D = 600
H = 75
P = 128
CHUNK = 120
NCHUNK = 5  # 5 * 120 = 600
R = 256  # rows per group (2 partition-blocks)
STAGE_GROUPS = 32  # groups per output stage buffer (32*256 = 8192 scores)

F32 = mybir.dt.float32
BF16 = mybir.dt.bfloat16


def build_nc(n_groups=None):
    """Build the per-core Bass program. n_groups limits work for dev testing."""
    nc = bacc.Bacc("TRN2", target_bir_lowering=False, debug=False,
                   num_devices=N_CORES)
    x = nc.declare_dram_parameter("x", [ROWS, D], F32, isOutput=False)
    w1t = nc.declare_dram_parameter("w1t", [D, H], F32, isOutput=False)
    b1 = nc.declare_dram_parameter("b1", [H, 1], F32, isOutput=False)
    w2t = nc.declare_dram_parameter("w2t", [H, 1], F32, isOutput=False)
    out = nc.declare_dram_parameter("out", [1, ROWS], F32, isOutput=True)

    n_full = ROWS // R  # 244
    starts = [g * R for g in range(n_full)]
    if ROWS % R:
        starts.append(ROWS - R)  # tail group overlaps; writes identical values
    if n_groups is not None:
        starts = starts[:n_groups]

    # xT free-layout order: block index = k * 2 + j  (k = feat chunk, j = row
    # half), so chunk k for the matmul is a contiguous 256-wide slice.
    blocks = [(k, j) for k in range(NCHUNK) for j in range(2)]

    with tile.TileContext(nc) as tc:
        with (
            tc.tile_pool(name="const", bufs=1) as const_pool,
            tc.tile_pool(name="sb", bufs=3) as sb,
            tc.tile_pool(name="stage", bufs=2) as stage_pool,
            tc.tile_pool(name="tp_ps", bufs=3, space="PSUM") as tp_ps,
            tc.tile_pool(name="h_ps", bufs=2, space="PSUM") as h_ps,
            tc.tile_pool(name="sc_ps", bufs=2, space="PSUM") as sc_ps,
        ):
            ident = const_pool.tile([P, P], F32)
            make_identity(nc, ident[:])

            w1t_f = const_pool.tile([CHUNK, NCHUNK, H], F32)
            nc.sync.dma_start(
                out=w1t_f[:], in_=w1t.rearrange("(k p) j -> p k j", p=CHUNK)
            )
            w1t_bf = const_pool.tile([CHUNK, NCHUNK, H], BF16)
            nc.vector.tensor_copy(out=w1t_bf[:], in_=w1t_f[:])

            b1_sb = const_pool.tile([H, 1], F32)
            nc.sync.dma_start(out=b1_sb[:], in_=b1[:])

            w2t_f = const_pool.tile([H, 1], F32)
            nc.sync.dma_start(out=w2t_f[:], in_=w2t[:])
            w2t_bf = const_pool.tile([H, 1], BF16)
            nc.vector.tensor_copy(out=w2t_bf[:], in_=w2t_f[:])

            stage = None
            stage_runs = []  # (stage_off_elems, dram_start_row)

            def flush_stage():
                nonlocal stage, stage_runs
                if stage is None:
                    return
                # merge consecutive runs (same stage+dram contiguity)
                merged = []
                for off, ds in stage_runs:
                    if merged and merged[-1][0] + merged[-1][2] == off and \
                            merged[-1][1] + merged[-1][2] == ds:
                        merged[-1][2] += R
                    else:
                        merged.append([off, ds, R])
                for off, ds, ln in merged:
                    nc.sync.dma_start(
                        out=out[0:1, ds:ds + ln], in_=stage[0:1, off:off + ln]
                    )
                stage = None
                stage_runs = []

            for gi, s in enumerate(starts):
                xg = sb.tile([P, 2, D], F32, tag="xg")
                nc.sync.dma_start(
                    out=xg[:],
                    in_=x[s:s + R].rearrange("(j p) f -> p j f", p=P),
                )

                xT = sb.tile([CHUNK, NCHUNK, 2, P], BF16, tag="xT")
                xT_flat = xT.rearrange("p k j c -> p (k j c)")

                # PE transposes into PSUM, packed 4/4/2 blocks per bank
                tp_groups = [blocks[0:4], blocks[4:8], blocks[8:10]]
                copy_engines = [nc.scalar, nc.vector, nc.vector]
                foff = 0
                for grp, eng in zip(tp_groups, copy_engines):
                    tp = tp_ps.tile([CHUNK, P * len(grp)], F32, tag="tp")
                    for bi, (k, j) in enumerate(grp):
                        nc.tensor.transpose(
                            tp[:, bi * P:(bi + 1) * P],
                            xg[:, j, k * CHUNK:(k + 1) * CHUNK],
                            ident[:],
                        )
                    sz = P * len(grp)
                    if eng is nc.scalar:
                        nc.scalar.copy(
                            out=xT_flat[:, foff:foff + sz], in_=tp[:]
                        )
                    else:
                        nc.vector.tensor_copy(
                            out=xT_flat[:, foff:foff + sz], in_=tp[:]
                        )
                    foff += sz

                # layer 1: h^T[75, 256] accumulated over 5 K-chunks
                hT = h_ps.tile([H, R], F32, tag="hT")
                for k in range(NCHUNK):
                    nc.tensor.matmul(
                        hT[:],
                        w1t_bf[:, k],
                        xT[:, k],
                        start=(k == 0),
                        stop=(k == NCHUNK - 1),
                    )

                h_sb = sb.tile([H, R], BF16, tag="h")
                nc.scalar.activation(
                    out=h_sb[:], in_=hT[:],
                    func=mybir.ActivationFunctionType.Relu, bias=b1_sb[:],
                )

                # layer 2: scores [1, 256]
                sc = sc_ps.tile([1, R], F32, tag="sc")
                nc.tensor.matmul(
                    sc[:], w2t_bf[:], h_sb[:], start=True, stop=True
                )

                if stage is None:
                    stage = stage_pool.tile([1, STAGE_GROUPS * R], F32,
                                            tag="st")
                off = len(stage_runs) * R
                nc.vector.tensor_copy(out=stage[0:1, off:off + R], in_=sc[:])
                stage_runs.append((off, s))
                if len(stage_runs) == STAGE_GROUPS or gi == len(starts) - 1:
                    flush_stage()

    nc.compile()
    return nc


_NC_CACHE = {}


def _get_nc(n_groups=None):
    if n_groups not in _NC_CACHE:
        _NC_CACHE[n_groups] = build_nc(n_groups)
    return _NC_CACHE[n_groups]


def make_in_maps(answer_vector, W1, b1, W2):
    w1t = np.ascontiguousarray(W1.T, dtype=np.float32)  # [600, 75]
    b1c = np.ascontiguousarray(np.asarray(b1, dtype=np.float32).reshape(H, 1))
    w2t = np.ascontiguousarray(np.asarray(W2, dtype=np.float32).reshape(H, 1))
    in_maps = []
    for i in range(N_CORES):
        shard = np.ascontiguousarray(
            answer_vector[:, i * BSHARD:(i + 1) * BSHARD, :], dtype=np.float32
        ).reshape(ROWS, D)
        in_maps.append({"x": shard, "w1t": w1t, "b1": b1c, "w2t": w2t})
    return in_maps


def assemble(results, b2):
    scores = np.empty((OPTIONS, BATCH), dtype=np.float32)
    for i in range(N_CORES):
        scores[:, i * BSHARD:(i + 1) * BSHARD] = \
            results[i]["out"].reshape(OPTIONS, BSHARD)
    scores += np.float32(np.asarray(b2).reshape(-1)[0])
    return np.ascontiguousarray(scores.T)


def run_on_hw(answer_vector, W1, b1, W2, b2, trace=False):
    nc = _get_nc()
    in_maps = make_in_maps(answer_vector, W1, b1, W2)
    res = run_bass_kernel_spmd(
        nc, in_maps, core_ids=list(range(N_CORES)), trace=trace
    )
    return assemble(res.results, b2), res


def kernel(answer_vector, W1, b1, W2, b2):
    out, _ = run_on_hw(answer_vector, W1, b1, W2, b2, trace=False)
    return out
